# revision 6
# baseline (speedup 1.0000x reference)
"""MoE block (AdaptFormer adapters, top-2 of 8 experts) on 8 TRN2 NeuronCores.

Data-parallel over the 8192 tokens (1024/core), router + expert adapter
weights replicated. Per core, one fused streaming pipeline (no phase split):

  - x ships as an exact bf16 hi/lo split, pre-transposed on the host to
    [D, T] and loaded per 512-token block so compute starts early.
  - logits: two accumulation groups per block -> lt_ps [24, 512]:
    rows 0:16 = xh @ [wgh|wgl] (one 16-col pass), rows 16:24 = xl @ wgh.
    Error ~3e-6, far below the 3.6e-5 min top-2/3 logit gap.
  - gating: per 128-token tile, PE-transpose the 24 logit rows, DVE-add
    the three partial columns, top-2 softmax (x0.5 adapter scale folded
    into the gates), PE-transpose gates back to g2T [8, tok] f32r.
  - experts run densely in bf16 (error budget is 2e-2; bf16 lands ~2e-3):
    HT chunks [128-of-512, 512] = Wd^T x (bf16), relu -> r (bf16),
    GB = Eblk^T @ g2T expands gates across the 512-wide expert axis,
    hg = r * GB (bf16), out tiles = hg-slices @ Wu (bf16) accumulated
    over the expert axis, stored as bf16 and widened to f32 on the host.
  - DMA is staged via explicit deps so the first block's inputs arrive
    first instead of all transfers sharing bandwidth concurrently.
All experts computed densely; sparse gates zero the non-top-2 terms
(mathematically identical to dispatch/combine).
"""
import numpy as np
import ml_dtypes
from contextlib import ExitStack

import concourse.bass as bass
import concourse.tile as tile
from concourse.tile import add_dep_helper
from concourse import bacc, mybir
from concourse.bass_utils import run_bass_kernel_spmd

N_CORES = 8
B_DIM, S_DIM, D = 2, 4096, 1024
T = B_DIM * S_DIM          # 8192 tokens
TC = T // N_CORES          # 1024 tokens per core
E, BK = 8, 64              # experts, bottleneck
EB = E * BK                # 512 concatenated expert axis
P = 128
KC = D // P                # D chunks
BC = EB // P               # bottleneck chunks
LBLK = 512                 # token block
NLB = TC // LBLK           # 2 blocks per core
TPB = LBLK // P            # token tiles per block
SCALE = 0.5
N_WARM = 9                 # PE warm-up matmuls during initial DMA wait
G = 2 * E                  # 16 = width of the combined [wgh|wgl] pass
XB = 32                    # xl-pass rows base (matmul out partition must be 0/32/64)
LW = XB + E                # 40 = logit psum rows (combined + xl pass)

F32 = mybir.dt.float32
F32R = mybir.dt.float32r
BF16 = mybir.dt.bfloat16
AL = mybir.AluOpType
ACTF = mybir.ActivationFunctionType
AX = mybir.AxisListType

_BUILD_CACHE = {}


def _build(include_bd: bool, include_bu: bool, reps: int = 1):
    key = (include_bd, include_bu, reps)
    if key in _BUILD_CACHE:
        return _BUILD_CACHE[key]

    nc = bacc.Bacc("TRN2", target_bir_lowering=False, debug=False,
                   num_devices=N_CORES)
    # x halves, pre-transposed: [D, TC] bf16
    xh_d = nc.dram_tensor("xh", [D, TC], BF16, kind="ExternalInput").ap()
    xl_d = nc.dram_tensor("xl", [D, TC], BF16, kind="ExternalInput").ap()
    wd_d = nc.dram_tensor("wd", [D, EB], BF16, kind="ExternalInput").ap()
    wu_d = nc.dram_tensor("wu", [EB, D], BF16, kind="ExternalInput").ap()
    # [wgh | wgl] side by side: [D, 16] bf16
    wg_d = nc.dram_tensor("wghl", [D, G], BF16, kind="ExternalInput").ap()
    id_d = nc.dram_tensor("ident", [P, P], F32, kind="ExternalInput").ap()
    eb_d = nc.dram_tensor("eblk", [E, EB], F32, kind="ExternalInput").ap()
    if include_bd:
        bd_d = nc.dram_tensor("bd", [P, BC], F32, kind="ExternalInput").ap()
    if include_bu:
        bu_d = nc.dram_tensor("bu", [E, D], BF16, kind="ExternalInput").ap()
    out_d = nc.dram_tensor("out", [TC, D], BF16, kind="ExternalOutput").ap()

    with tile.TileContext(nc) as tc, ExitStack() as ctx:
        wpool = ctx.enter_context(tc.tile_pool(name="weights", bufs=1))
        hgpool = ctx.enter_context(tc.tile_pool(name="hg", bufs=6))
        rpool = ctx.enter_context(tc.tile_pool(name="relu", bufs=2))
        gpool = ctx.enter_context(tc.tile_pool(name="gates", bufs=2))
        opool = ctx.enter_context(tc.tile_pool(name="osb", bufs=3))

        ht_ps_pool = ctx.enter_context(
            tc.tile_pool(name="htps", bufs=2, space="PSUM"))
        lt_ps_pool = ctx.enter_context(
            tc.tile_pool(name="ltps", bufs=2, space="PSUM"))
        small_ps_pool = ctx.enter_context(
            tc.tile_pool(name="smps", bufs=1, space="PSUM"))
        gb_ps_pool = ctx.enter_context(
            tc.tile_pool(name="gbps", bufs=1, space="PSUM"))
        o_ps_pool = ctx.enter_context(
            tc.tile_pool(name="ops", bufs=2, space="PSUM"))

        # PE warm-up first: source tile is memset (no DMA wait); keeps the
        # HAM clock un-throttled while the first block's inputs stream in.
        warm32 = wpool.tile([P, LBLK], F32, tag="warm32")
        nc.vector.memset(warm32[:], 0.001)
        warm_src = wpool.tile([P, LBLK], F32R, tag="warmsrc")
        nc.vector.tensor_copy(warm_src[:], warm32[:])
        warm_ps = gb_ps_pool.tile([P, LBLK], F32R, tag="gbps")
        for i in range(N_WARM):
            nc.tensor.matmul(warm_ps[:].bitcast(F32), warm_src[:, 0:P],
                             warm_src[:], start=True, stop=True)

        # ---- staged DMA: stage A = consts + first x block ----
        wg_sb = wpool.tile([P, KC, G], BF16, tag="wghl")
        nc.sync.dma_start(wg_sb[:], wg_d.rearrange("(c p) n -> p c n", p=P))
        ident = wpool.tile([P, P], F32, tag="ident")
        nc.sync.dma_start(ident[:], id_d)
        ident_r = wpool.tile([P, P], F32R, tag="identr")
        nc.sync.dma_start(ident_r[:], id_d.bitcast(F32R))
        eblk = wpool.tile([E, EB], F32R, tag="eblk")
        nc.sync.dma_start(eblk[:], eb_d.bitcast(F32R))
        if include_bd:
            bd_sb = wpool.tile([P, BC], F32, tag="bd")
            nc.sync.dma_start(bd_sb[:], bd_d)
        if include_bu:
            bu_sb = wpool.tile([E, D], BF16, tag="bu")
            nc.sync.dma_start(bu_sb[:], bu_d)

        xh_r = xh_d.rearrange("(c p) t -> p c t", p=P)
        xl_r = xl_d.rearrange("(c p) t -> p c t", p=P)
        xh_sb = [wpool.tile([P, KC, LBLK], BF16, tag=f"xh{b}", name=f"xh{b}")
                 for b in range(NLB)]
        xl_sb = [wpool.tile([P, KC, LBLK], BF16, tag=f"xl{b}", name=f"xl{b}")
                 for b in range(NLB)]
        xh0_i = nc.sync.dma_start(xh_sb[0][:], xh_r[:, :, 0:LBLK])

        def dma_after(dst, src, prev):
            i = nc.sync.dma_start(dst, src)
            add_dep_helper(i.ins, prev.ins, sync=True,
                           reason="staged dma priority")
            return i

        # stage B: adapter down-proj weights + second x block
        wd_sb = wpool.tile([P, KC, EB], BF16, tag="wd")
        wd_i = dma_after(wd_sb[:], wd_d.rearrange("(c p) n -> p c n", p=P),
                         xh0_i)
        xh1_i = dma_after(xh_sb[1][:], xh_r[:, :, LBLK:TC], xh0_i)
        # stage C: first xl block + up-proj weights
        xl0_i = dma_after(xl_sb[0][:], xl_r[:, :, 0:LBLK], xh1_i)
        wu_sb = wpool.tile([P, BC, D], BF16, tag="wu")
        dma_after(wu_sb[:], wu_d.rearrange("(c p) n -> p c n", p=P), wd_i)
        # stage D: second xl block
        dma_after(xl_sb[1][:], xl_r[:, :, LBLK:TC], xl0_i)

        def emit_logits_c(blk):
            """Combined [wgh|wgl] pass -> lt_ps rows 0:16."""
            lt_ps = lt_ps_pool.tile([LW, LBLK], F32, tag="ltps",
                                    name=f"lt{blk}")
            for c in range(KC):
                nc.tensor.matmul(lt_ps[0:G, :], wg_sb[:, c, :],
                                 xh_sb[blk][:, c, :],
                                 start=(c == 0), stop=(c == KC - 1))
            return lt_ps

        def emit_logits_xl(blk, lt_ps):
            """xl @ wgh pass -> lt_ps rows 16:24, then copy to SBUF."""
            for c in range(KC):
                nc.tensor.matmul(lt_ps[XB:LW, :], wg_sb[:, c, 0:E],
                                 xl_sb[blk][:, c, :],
                                 start=(c == 0), stop=(c == KC - 1))
            lt_sb = gpool.tile([LW, LBLK], F32, tag="ltsb")
            nc.scalar.copy(lt_sb[:], lt_ps[:])
            return lt_sb

        def emit_ltT(lt_sb, bo):
            """Transpose 24 logit rows of one 128-token tile into PSUM."""
            small = small_ps_pool.tile([P, LW + P], F32, tag="smps")
            nc.tensor.transpose(small[:, 0:LW], lt_sb[:, bass.ts(bo, P)],
                                ident[0:LW, 0:LW])
            return small

        def emit_chain(small):
            """Top-2 softmax (x0.5) for one 128-token tile; g2 -> f32r."""
            l24 = gpool.tile([P, LW], F32, tag="l24")
            nc.scalar.copy(l24[:], small[:, 0:LW])
            l_s = gpool.tile([P, E], F32, tag="lpart")
            nc.vector.tensor_tensor(l_s[:], l24[:, 0:E], l24[:, E:G],
                                    op=AL.add)
            l_sb = gpool.tile([P, E], F32, tag="lsb")
            nc.vector.tensor_tensor(l_sb[:], l_s[:], l24[:, XB:LW],
                                    op=AL.add)
            m1 = gpool.tile([P, 1], F32, tag="m1")
            nc.vector.tensor_reduce(m1[:], l_sb[:], AX.X, AL.max)
            m1n = gpool.tile([P, 1], F32, tag="m1n")
            nc.vector.tensor_scalar_mul(m1n[:], m1[:], -1.0)
            mask1 = gpool.tile([P, E], F32, tag="mask1")
            nc.vector.tensor_scalar(mask1[:], l_sb[:], m1[:], None,
                                    op0=AL.is_ge)
            lm = gpool.tile([P, E], F32, tag="lm")
            nc.vector.scalar_tensor_tensor(
                lm[:], mask1[:], -1e30, l_sb[:], op0=AL.mult, op1=AL.add)
            m2 = gpool.tile([P, 1], F32, tag="m2")
            nc.vector.tensor_reduce(m2[:], lm[:], AX.X, AL.max)
            e2 = gpool.tile([P, 1], F32, tag="e2")
            nc.scalar.activation(e2[:], m2[:], ACTF.Exp, bias=m1n[:])
            d2 = gpool.tile([P, 1], F32, tag="d2")
            nc.scalar.activation(d2[:], e2[:], ACTF.Copy,
                                 bias=1.0 / SCALE, scale=1.0 / SCALE)
            rh = gpool.tile([P, 1], F32, tag="rh")
            nc.vector.reciprocal(rh[:], d2[:])
            expl = gpool.tile([P, E], F32, tag="expl")
            nc.scalar.activation(expl[:], l_sb[:], ACTF.Exp, bias=m1n[:])
            mask2 = gpool.tile([P, E], F32, tag="mask2")
            nc.vector.tensor_scalar(mask2[:], l_sb[:], m2[:], None,
                                    op0=AL.is_ge)
            g2 = gpool.tile([P, E], F32, tag="g2")
            nc.vector.scalar_tensor_tensor(
                g2[:], expl[:], rh[:], mask2[:], op0=AL.mult, op1=AL.mult)
            g2r = gpool.tile([P, E], F32R, tag="g2r")
            nc.vector.tensor_copy(g2r[:], g2[:])
            return g2r

        def emit_g2T(small, g2r, g2t_blk, bo):
            """Transpose gates back to [8, tok] f32r and copy to SBUF."""
            g2t_ps = small[0:E, LW:LW + P].bitcast(F32R)
            nc.tensor.transpose(g2t_ps, g2r[:], ident_r[:])
            nc.scalar.copy(g2t_blk[:, bass.ts(bo, P)], g2t_ps)

        def emit_ht(blk, k):
            """HT chunk k: relu(Wd^T x) in bf16."""
            ht_ps = ht_ps_pool.tile([P, LBLK], F32, tag="htps")
            for c in range(KC):
                nc.tensor.matmul(ht_ps[:], wd_sb[:, c, bass.ts(k, P)],
                                 xh_sb[blk][:, c, :],
                                 start=(c == 0), stop=(c == KC - 1))
            r_k = rpool.tile([P, LBLK], BF16, tag="relu")
            if include_bd:
                nc.scalar.activation(r_k[:], ht_ps[:], ACTF.Relu,
                                     bias=bd_sb[:, k:k + 1])
            else:
                nc.scalar.activation(r_k[:], ht_ps[:], ACTF.Relu)
            return r_k

        def emit_gb_mult(blk, k, g2t_blk, r_k):
            """Gate-expand matmul + hg = relu * gates (bf16)."""
            gb_ps = gb_ps_pool.tile([P, LBLK], F32R, tag="gbps")
            nc.tensor.matmul(gb_ps[:].bitcast(F32), eblk[:, bass.ts(k, P)],
                             g2t_blk[:], start=True, stop=True)
            hg_k = hgpool.tile([P, LBLK], BF16, tag="hg",
                               name=f"hg{blk}_{k}")
            nc.vector.tensor_tensor(hg_k[:], r_k[:], gb_ps[:].bitcast(F32),
                                    op=AL.mult)
            return hg_k

        def emit_out(blk, hgs, g2t_bf):
            """out tiles = HG @ Wu (+ g2 @ bu), stored bf16."""
            for bo in range(TPB):
                t = blk * TPB + bo
                rows = bass.ts(t, P)
                tok = bass.ts(bo, P)
                for h in range(2):
                    o_ps = o_ps_pool.tile([P, 512], F32, tag="ops")
                    for k in range(BC):
                        nc.tensor.matmul(
                            o_ps[:], hgs[k][:, tok],
                            wu_sb[:, k, bass.ts(h, 512)],
                            start=(k == 0),
                            stop=(k == BC - 1 and not include_bu))
                    if include_bu:
                        nc.tensor.matmul(o_ps[:], g2t_bf[:, tok],
                                         bu_sb[:, bass.ts(h, 512)],
                                         start=False, stop=True)
                    o_sb = opool.tile([P, 512], BF16, tag="osb")
                    if h == 0:
                        nc.vector.tensor_copy(o_sb[:], o_ps[:])
                    else:
                        nc.scalar.copy(o_sb[:], o_ps[:])
                    nc.scalar.dma_start(out_d[rows, bass.ts(h, 512)],
                                        o_sb[:])

        for rep in range(reps):
            # logits for both blocks early: fills the PE while wd/xl stream
            lt0 = emit_logits_c(0)
            lt1 = emit_logits_c(1)

            # ---- block 0 ----
            lt_sb0 = emit_logits_xl(0, lt0)
            g2t0 = gpool.tile([E, LBLK], F32R, tag="g2t", name="g2t0")
            smalls0 = [emit_ltT(lt_sb0, bo) for bo in range(TPB)]
            g2rs0 = [emit_chain(smalls0[bo]) for bo in range(TPB)]
            r0 = [emit_ht(0, k) for k in range(BC)]
            for bo in range(TPB):
                emit_g2T(smalls0[bo], g2rs0[bo], g2t0, bo)
            g2t_bf0 = None
            if include_bu:
                g2t_bf0 = gpool.tile([E, LBLK], BF16, tag="g2tb",
                                     name="g2tb0")
                nc.vector.tensor_copy(g2t_bf0[:], g2t0[:])
            hgs0 = [emit_gb_mult(0, k, g2t0, r0[k]) for k in range(BC)]

            # ---- block 1 gating interleaved with block 0 output ----
            lt_sb1 = emit_logits_xl(1, lt1)
            smalls1 = [emit_ltT(lt_sb1, bo) for bo in range(TPB)]
            g2rs1 = [emit_chain(smalls1[bo]) for bo in range(TPB)]
            emit_out(0, hgs0, g2t_bf0)
            g2t1 = gpool.tile([E, LBLK], F32R, tag="g2t", name="g2t1")
            for bo in range(TPB):
                emit_g2T(smalls1[bo], g2rs1[bo], g2t1, bo)
            g2t_bf1 = None
            if include_bu:
                g2t_bf1 = gpool.tile([E, LBLK], BF16, tag="g2tb",
                                     name="g2tb1")
                nc.vector.tensor_copy(g2t_bf1[:], g2t1[:])
            r1 = [emit_ht(1, k) for k in range(BC)]
            hgs1 = [emit_gb_mult(1, k, g2t1, r1[k]) for k in range(BC)]
            emit_out(1, hgs1, g2t_bf1)

    nc.compile()
    _BUILD_CACHE[key] = nc
    return nc


def _split_bf16(a):
    hi = a.astype(ml_dtypes.bfloat16)
    lo = (a - hi.astype(np.float32)).astype(ml_dtypes.bfloat16)
    return hi, lo


def kernel(x, w_gate, w_noise, Wd, bd, Wu, bu, reps: int = 1):
    x = np.ascontiguousarray(np.asarray(x, dtype=np.float32))
    assert x.shape == (B_DIM, S_DIM, D), x.shape
    wg = np.ascontiguousarray(np.asarray(w_gate, dtype=np.float32))
    Wd = np.asarray(Wd, dtype=np.float32)
    Wu = np.asarray(Wu, dtype=np.float32)
    bd = np.asarray(bd, dtype=np.float32)
    bu = np.asarray(bu, dtype=np.float32)

    include_bd = bool(np.any(bd))
    include_bu = bool(np.any(bu))
    nc = _build(include_bd, include_bu, reps)

    xf = x.reshape(T, D)
    xh, xl = _split_bf16(xf)
    xht_full = np.ascontiguousarray(xh.T)   # [D, T]
    xlt_full = np.ascontiguousarray(xl.T)
    wgh, wgl = _split_bf16(wg)
    wghl = np.ascontiguousarray(
        np.concatenate([wgh, wgl], axis=1))            # [D, 16] bf16
    wd_all = np.ascontiguousarray(
        Wd.transpose(1, 0, 2).reshape(D, EB)).astype(ml_dtypes.bfloat16)
    wu_flat = np.ascontiguousarray(
        Wu.reshape(EB, D)).astype(ml_dtypes.bfloat16)
    ident = np.eye(P, dtype=np.float32)
    eblk = np.kron(np.eye(E, dtype=np.float32),
                   np.ones((1, BK), dtype=np.float32))  # [E, EB]

    shared = dict(wd=wd_all, wu=wu_flat, wghl=wghl, ident=ident, eblk=eblk)
    if include_bd:
        # [P, BC] partition-major per chunk: bd_sb[p, k] = bd_flat[128k+p]
        shared["bd"] = np.ascontiguousarray(
            bd.reshape(EB)[np.arange(P)[:, None] + P * np.arange(BC)[None]])
    if include_bu:
        shared["bu"] = np.ascontiguousarray(bu).astype(ml_dtypes.bfloat16)

    in_maps = []
    for c in range(N_CORES):
        sl = slice(c * TC, (c + 1) * TC)
        in_maps.append(dict(xh=np.ascontiguousarray(xht_full[:, sl]),
                            xl=np.ascontiguousarray(xlt_full[:, sl]),
                            **shared))
    kernel.last_in_maps = in_maps
    res = run_bass_kernel_spmd(nc, in_maps, core_ids=list(range(N_CORES)))
    out = np.concatenate([res.results[c]["out"].astype(np.float32)
                          for c in range(N_CORES)], axis=0)
    return out.reshape(B_DIM, S_DIM, D)


# revision 8
# speedup vs baseline: 1.1657x; 1.1657x over previous
"""MoE block (AdaptFormer adapters, top-2 of 8 experts) on 8 TRN2 NeuronCores.

Data-parallel over the 8192 tokens (1024/core), router + expert adapter
weights replicated. Per core, one fused streaming pipeline (no phase split):

  - x ships as an exact bf16 hi/lo split, pre-transposed on the host to
    [D, T]; DMA is a fully serialized priority chain (xh block 0 first)
    so early compute is never starved by concurrent transfers.
  - logits per 512-token block: two accumulation groups -> lt_ps [40, 512]:
    rows 0:16 = xh @ [wgh|wgl] (one 16-col pass), rows 32:40 = xl @ wgh.
    Error ~3e-6, far below the 3.6e-5 min top-2/3 logit gap.
  - gating is batched per block: 4 PE transposes land the 40 logit rows in
    one [128, 4, 40] psum tile; the top-2 softmax (x0.5 adapter scale
    folded in) runs as ~16 DVE/ACT ops on [128, 4, 8] strided views
    (broadcast_to for the per-token max/denominator), then 4 PE
    transposes produce g2T [8, 512] bf16.
  - experts run densely in bf16 (error budget 2e-2; bf16 lands ~4e-3):
    HT chunks = Wd^T x -> relu (bf16), GB = Eblk^T @ g2T expands gates
    across the 512-wide expert axis, hg = relu * GB (bf16), out tiles =
    hg @ Wu accumulated over the expert axis, stored bf16, widened on
    the host.
All experts computed densely; sparse gates zero the non-top-2 terms
(mathematically identical to dispatch/combine).
"""
import numpy as np
import ml_dtypes
from contextlib import ExitStack

import concourse.bass as bass
import concourse.tile as tile
from concourse.tile import add_dep_helper
from concourse import bacc, mybir
from concourse.bass_utils import run_bass_kernel_spmd

N_CORES = 8
B_DIM, S_DIM, D = 2, 4096, 1024
T = B_DIM * S_DIM          # 8192 tokens
TC = T // N_CORES          # 1024 tokens per core
E, BK = 8, 64              # experts, bottleneck
EB = E * BK                # 512 concatenated expert axis
P = 128
KC = D // P                # D chunks
HC = KC // 2               # half of the D chunks (split xh block-0 DMA)
BC = EB // P               # bottleneck chunks
LBLK = 512                 # token block
NLB = TC // LBLK           # 2 blocks per core
TPB = LBLK // P            # token tiles per block
SCALE = 0.5
N_WARM = 4                 # PE warm-up matmuls during initial DMA wait
N_FILL = 4                 # PE fillers while wd streams in
G = 2 * E                  # 16 = width of the combined [wgh|wgl] pass
XB = 32                    # xl-pass rows base (matmul out partition 0/32/64)
LW = XB + E                # 40 = logit psum rows (combined + xl pass)

F32 = mybir.dt.float32
F32R = mybir.dt.float32r
BF16 = mybir.dt.bfloat16
AL = mybir.AluOpType
ACTF = mybir.ActivationFunctionType
AX = mybir.AxisListType

_BUILD_CACHE = {}


def _build(include_bd: bool, include_bu: bool, reps: int = 1):
    key = (include_bd, include_bu, reps)
    if key in _BUILD_CACHE:
        return _BUILD_CACHE[key]

    nc = bacc.Bacc("TRN2", target_bir_lowering=False, debug=False,
                   num_devices=N_CORES)
    # x halves, pre-transposed: [D, TC] bf16
    xh_d = nc.dram_tensor("xh", [D, TC], BF16, kind="ExternalInput").ap()
    xl_d = nc.dram_tensor("xl", [D, TC], BF16, kind="ExternalInput").ap()
    wd_d = nc.dram_tensor("wd", [D, EB], BF16, kind="ExternalInput").ap()
    wu_d = nc.dram_tensor("wu", [EB, D], BF16, kind="ExternalInput").ap()
    # [wgh | wgl] side by side: [D, 16] bf16
    wg_d = nc.dram_tensor("wghl", [D, G], BF16, kind="ExternalInput").ap()
    id_d = nc.dram_tensor("ident", [P, P], F32, kind="ExternalInput").ap()
    idb_d = nc.dram_tensor("identb", [P, P], BF16, kind="ExternalInput").ap()
    eb_d = nc.dram_tensor("eblk", [E, EB], BF16, kind="ExternalInput").ap()
    if include_bd:
        bd_d = nc.dram_tensor("bd", [P, BC], F32, kind="ExternalInput").ap()
    if include_bu:
        bu_d = nc.dram_tensor("bu", [E, D], BF16, kind="ExternalInput").ap()
    out_d = nc.dram_tensor("out", [TC, D], BF16, kind="ExternalOutput").ap()

    with tile.TileContext(nc) as tc, ExitStack() as ctx:
        wpool = ctx.enter_context(tc.tile_pool(name="weights", bufs=1))
        hgpool = ctx.enter_context(tc.tile_pool(name="hg", bufs=8))
        rpool = ctx.enter_context(tc.tile_pool(name="relu", bufs=8))
        gpool = ctx.enter_context(tc.tile_pool(name="gates", bufs=2))
        opool = ctx.enter_context(tc.tile_pool(name="osb", bufs=3))

        htgb_ps_pool = ctx.enter_context(
            tc.tile_pool(name="htgb", bufs=2, space="PSUM"))
        lt_ps_pool = ctx.enter_context(
            tc.tile_pool(name="ltps", bufs=2, space="PSUM"))
        small_ps_pool = ctx.enter_context(
            tc.tile_pool(name="smps", bufs=1, space="PSUM"))
        g2t_ps_pool = ctx.enter_context(
            tc.tile_pool(name="g2tps", bufs=1, space="PSUM"))
        o_ps_pool = ctx.enter_context(
            tc.tile_pool(name="ops", bufs=2, space="PSUM"))

        # PE warm-up first: source tile is memset (no DMA wait); keeps the
        # HAM clock un-throttled while the first block's inputs stream in.
        warm32 = wpool.tile([P, LBLK], F32, tag="warm32")
        nc.vector.memset(warm32[:], 0.001)
        warm_src = wpool.tile([P, LBLK], BF16, tag="warmsrc")
        nc.vector.tensor_copy(warm_src[:], warm32[:])
        warm_ps = o_ps_pool.tile([P, LBLK], F32, tag="ops")

        def emit_warm(n):
            for _ in range(n):
                nc.tensor.matmul(warm_ps[:], warm_src[:, 0:P], warm_src[:],
                                 start=True, stop=True)

        emit_warm(N_WARM)

        # ---- fully serialized priority DMA chain on the sync queue ----
        prev = [None]

        def dma(dst, src):
            i = nc.sync.dma_start(dst, src)
            if prev[0] is not None:
                add_dep_helper(i.ins, prev[0].ins, sync=True,
                               reason="dma priority chain")
            prev[0] = i
            return i

        wg_sb = wpool.tile([P, KC, G], BF16, tag="wghl")
        dma(wg_sb[:], wg_d.rearrange("(c p) n -> p c n", p=P))

        xh_r = xh_d.rearrange("(c p) t -> p c t", p=P)
        xl_r = xl_d.rearrange("(c p) t -> p c t", p=P)
        # xh per (block, half-of-D): 4 tiles so compute starts on the
        # first 512KB
        xh_sb = [[wpool.tile([P, HC, LBLK], BF16, tag=f"xh{b}{h}",
                             name=f"xh{b}{h}") for h in range(2)]
                 for b in range(NLB)]
        xl_sb = [wpool.tile([P, KC, LBLK], BF16, tag=f"xl{b}", name=f"xl{b}")
                 for b in range(NLB)]

        def xh_c(b, c):
            return xh_sb[b][c // HC][:, c % HC, :]

        dma(xh_sb[0][0][:], xh_r[:, 0:HC, 0:LBLK])
        dma(xh_sb[0][1][:], xh_r[:, HC:KC, 0:LBLK])
        wd_sb = wpool.tile([P, KC, EB], BF16, tag="wd")
        dma(wd_sb[:], wd_d.rearrange("(c p) n -> p c n", p=P))
        dma(xh_sb[1][0][:], xh_r[:, 0:HC, LBLK:TC])
        dma(xh_sb[1][1][:], xh_r[:, HC:KC, LBLK:TC])
        dma(xl_sb[0][:], xl_r[:, :, 0:LBLK])
        ident = wpool.tile([P, P], F32, tag="ident")
        dma(ident[:], id_d)
        ident_b = wpool.tile([P, P], BF16, tag="identb")
        dma(ident_b[:], idb_d)
        eblk = wpool.tile([E, EB], BF16, tag="eblk")
        dma(eblk[:], eb_d)
        if include_bd:
            bd_sb = wpool.tile([P, BC], F32, tag="bd")
            dma(bd_sb[:], bd_d)
        if include_bu:
            bu_sb = wpool.tile([E, D], BF16, tag="bu")
            dma(bu_sb[:], bu_d)
        wu_sb = wpool.tile([P, BC, D], BF16, tag="wu")
        dma(wu_sb[:], wu_d.rearrange("(c p) n -> p c n", p=P))
        dma(xl_sb[1][:], xl_r[:, :, LBLK:TC])

        def emit_logits_c(blk):
            """Combined [wgh|wgl] pass -> lt_ps rows 0:16."""
            lt_ps = lt_ps_pool.tile([LW, LBLK], F32, tag="ltps",
                                    name=f"lt{blk}")
            for c in range(KC):
                nc.tensor.matmul(lt_ps[0:G, :], wg_sb[:, c, :], xh_c(blk, c),
                                 start=(c == 0), stop=(c == KC - 1))
            return lt_ps

        def emit_logits_xl(blk, lt_ps):
            """xl @ wgh pass -> lt_ps rows 32:40, then copy to SBUF."""
            for c in range(KC):
                nc.tensor.matmul(lt_ps[XB:LW, :], wg_sb[:, c, 0:E],
                                 xl_sb[blk][:, c, :],
                                 start=(c == 0), stop=(c == KC - 1))
            lt_sb = gpool.tile([LW, LBLK], F32, tag="ltsb")
            nc.scalar.copy(lt_sb[:], lt_ps[:])
            return lt_sb

        def emit_ltT(lt_sb):
            """4 transposes: logit rows for the whole block into PSUM."""
            small = small_ps_pool.tile([P, TPB, LW + 8], F32, tag="smps")
            for t in range(TPB):
                nc.tensor.transpose(small[:, t, 0:LW],
                                    lt_sb[:, bass.ts(t, P)],
                                    ident[0:LW, 0:LW])
            return small

        def emit_chain(small, blk):
            """Batched top-2 softmax (x0.5) for all 512 tokens of a block."""
            l24 = gpool.tile([P, TPB, LW], F32, tag="l24")
            nc.scalar.copy(l24[:], small[:, :, 0:LW])
            l_s = gpool.tile([P, TPB, E], F32, tag="lpart")
            nc.vector.tensor_tensor(l_s[:], l24[:, :, 0:E], l24[:, :, E:G],
                                    op=AL.add)
            l_sb = gpool.tile([P, TPB, E], F32, tag="lsb")
            nc.vector.tensor_tensor(l_sb[:], l_s[:], l24[:, :, XB:LW],
                                    op=AL.add)
            sh3 = [P, TPB, E]
            m1 = gpool.tile([P, TPB, 1], F32, tag="m1")
            nc.vector.tensor_reduce(m1[:, :, 0], l_sb[:], AX.X, AL.max)
            mask1 = gpool.tile(sh3, F32, tag="mask1")
            nc.vector.tensor_tensor(mask1[:], l_sb[:],
                                    m1[:].broadcast_to(sh3), op=AL.is_ge)
            lm = gpool.tile(sh3, F32, tag="lm")
            nc.vector.scalar_tensor_tensor(
                lm[:], mask1[:], -1e30, l_sb[:], op0=AL.mult, op1=AL.add)
            m2 = gpool.tile([P, TPB, 1], F32, tag="m2")
            nc.vector.tensor_reduce(m2[:, :, 0], lm[:], AX.X, AL.max)
            e2m = gpool.tile([P, TPB, 1], F32, tag="e2m")
            nc.vector.tensor_tensor(e2m[:], m2[:], m1[:], op=AL.subtract)
            e2 = gpool.tile([P, TPB, 1], F32, tag="e2")
            nc.scalar.activation(e2[:], e2m[:], ACTF.Exp)
            d2 = gpool.tile([P, TPB, 1], F32, tag="d2")
            nc.scalar.activation(d2[:], e2[:], ACTF.Copy,
                                 bias=1.0 / SCALE, scale=1.0 / SCALE)
            rh = gpool.tile([P, TPB, 1], F32, tag="rh")
            nc.vector.reciprocal(rh[:], d2[:])
            lsh = gpool.tile(sh3, F32, tag="lsh")
            nc.vector.tensor_tensor(lsh[:], l_sb[:],
                                    m1[:].broadcast_to(sh3), op=AL.subtract)
            expl = gpool.tile(sh3, F32, tag="expl")
            nc.scalar.activation(expl[:], lsh[:], ACTF.Exp)
            mask2 = gpool.tile(sh3, F32, tag="mask2")
            nc.vector.tensor_tensor(mask2[:], l_sb[:],
                                    m2[:].broadcast_to(sh3), op=AL.is_ge)
            t1 = gpool.tile(sh3, F32, tag="t1")
            nc.vector.tensor_tensor(t1[:], expl[:], mask2[:], op=AL.mult)
            g2 = gpool.tile(sh3, BF16, tag="g2", name=f"g2_{blk}")
            nc.vector.tensor_tensor(g2[:], t1[:],
                                    rh[:].broadcast_to(sh3), op=AL.mult)
            return g2

        def emit_g2T(g2):
            """4 transposes: gates back to [8, tok] bf16 in SBUF."""
            g2t_ps = g2t_ps_pool.tile([E, LBLK], BF16, tag="g2tps")
            for t in range(TPB):
                nc.tensor.transpose(g2t_ps[:, bass.ts(t, P)], g2[:, t, :],
                                    ident_b[:])
            g2t_sb = gpool.tile([E, LBLK], BF16, tag="g2t")
            nc.scalar.copy(g2t_sb[:], g2t_ps[:])
            return g2t_sb

        def emit_ht(blk, k):
            """HT chunk k: relu(Wd^T x) in bf16."""
            ht_ps = htgb_ps_pool.tile([P, LBLK], F32, tag="htps")
            for c in range(KC):
                nc.tensor.matmul(ht_ps[:], wd_sb[:, c, bass.ts(k, P)],
                                 xh_c(blk, c),
                                 start=(c == 0), stop=(c == KC - 1))
            r_k = rpool.tile([P, LBLK], BF16, tag="relu")
            if include_bd:
                nc.scalar.activation(r_k[:], ht_ps[:], ACTF.Relu,
                                     bias=bd_sb[:, k:k + 1])
            else:
                nc.scalar.activation(r_k[:], ht_ps[:], ACTF.Relu)
            return r_k

        def emit_gb(k, g2t_sb):
            """Gate-expand matmul for chunk k."""
            gb_ps = htgb_ps_pool.tile([P, LBLK], F32, tag="htps")
            nc.tensor.matmul(gb_ps[:], eblk[:, bass.ts(k, P)], g2t_sb[:],
                             start=True, stop=True)
            return gb_ps

        def emit_hg(blk, k, r_k, gb_ps):
            """hg = relu * gates (bf16, DVE)."""
            hg_k = hgpool.tile([P, LBLK], BF16, tag="hg",
                               name=f"hg{blk}_{k}")
            nc.vector.tensor_tensor(hg_k[:], r_k[:], gb_ps[:], op=AL.mult)
            return hg_k

        def emit_out(blk, hgs, g2t_sb):
            """out tiles = HG @ Wu (+ g2 @ bu), stored bf16."""
            for bo in range(TPB):
                t = blk * TPB + bo
                rows = bass.ts(t, P)
                tok = bass.ts(bo, P)
                for h in range(2):
                    o_ps = o_ps_pool.tile([P, 512], F32, tag="ops")
                    for k in range(BC):
                        nc.tensor.matmul(
                            o_ps[:], hgs[k][:, tok],
                            wu_sb[:, k, bass.ts(h, 512)],
                            start=(k == 0),
                            stop=(k == BC - 1 and not include_bu))
                    if include_bu:
                        nc.tensor.matmul(o_ps[:], g2t_sb[:, tok],
                                         bu_sb[:, bass.ts(h, 512)],
                                         start=False, stop=True)
                    o_sb = opool.tile([P, 512], BF16, tag="osb")
                    if h == 0:
                        nc.vector.tensor_copy(o_sb[:], o_ps[:])
                    else:
                        nc.scalar.copy(o_sb[:], o_ps[:])
                    nc.scalar.dma_start(out_d[rows, bass.ts(h, 512)],
                                        o_sb[:])

        for rep in range(reps):
            # ---- block 0: logits (streams behind the xh DMA), fillers ----
            lt0 = emit_logits_c(0)
            emit_warm(N_FILL)
            r0 = [emit_ht(0, k) for k in range(BC)]
            lt_sb0 = emit_logits_xl(0, lt0)
            small0 = emit_ltT(lt_sb0)
            g2_0 = emit_chain(small0, 0)
            g2t0 = emit_g2T(g2_0)
            gbs0 = [emit_gb(k, g2t0) for k in range(BC)]
            hgs0 = [emit_hg(0, k, r0[k], gbs0[k]) for k in range(BC)]

            # ---- block 1 gating interleaved with block 0 output ----
            lt1 = emit_logits_c(1)
            lt_sb1 = emit_logits_xl(1, lt1)
            small1 = emit_ltT(lt_sb1)
            g2_1 = emit_chain(small1, 1)
            emit_out(0, hgs0, g2t0)
            g2t1 = emit_g2T(g2_1)
            r1 = [emit_ht(1, k) for k in range(BC)]
            gbs1 = [emit_gb(k, g2t1) for k in range(BC)]
            hgs1 = [emit_hg(1, k, r1[k], gbs1[k]) for k in range(BC)]
            emit_out(1, hgs1, g2t1)

    nc.compile()
    _BUILD_CACHE[key] = nc
    return nc


def _split_bf16(a):
    hi = a.astype(ml_dtypes.bfloat16)
    lo = (a - hi.astype(np.float32)).astype(ml_dtypes.bfloat16)
    return hi, lo


def kernel(x, w_gate, w_noise, Wd, bd, Wu, bu, reps: int = 1):
    x = np.ascontiguousarray(np.asarray(x, dtype=np.float32))
    assert x.shape == (B_DIM, S_DIM, D), x.shape
    wg = np.ascontiguousarray(np.asarray(w_gate, dtype=np.float32))
    Wd = np.asarray(Wd, dtype=np.float32)
    Wu = np.asarray(Wu, dtype=np.float32)
    bd = np.asarray(bd, dtype=np.float32)
    bu = np.asarray(bu, dtype=np.float32)

    include_bd = bool(np.any(bd))
    include_bu = bool(np.any(bu))
    nc = _build(include_bd, include_bu, reps)

    xf = x.reshape(T, D)
    xh, xl = _split_bf16(xf)
    xht_full = np.ascontiguousarray(xh.T)   # [D, T]
    xlt_full = np.ascontiguousarray(xl.T)
    wgh, wgl = _split_bf16(wg)
    wghl = np.ascontiguousarray(
        np.concatenate([wgh, wgl], axis=1))            # [D, 16] bf16
    wd_all = np.ascontiguousarray(
        Wd.transpose(1, 0, 2).reshape(D, EB)).astype(ml_dtypes.bfloat16)
    wu_flat = np.ascontiguousarray(
        Wu.reshape(EB, D)).astype(ml_dtypes.bfloat16)
    ident = np.eye(P, dtype=np.float32)
    eblk = np.kron(np.eye(E, dtype=np.float32),
                   np.ones((1, BK), dtype=np.float32))  # [E, EB]

    shared = dict(wd=wd_all, wu=wu_flat, wghl=wghl, ident=ident,
                  identb=ident.astype(ml_dtypes.bfloat16),
                  eblk=eblk.astype(ml_dtypes.bfloat16))
    if include_bd:
        # [P, BC] partition-major per chunk: bd_sb[p, k] = bd_flat[128k+p]
        shared["bd"] = np.ascontiguousarray(
            bd.reshape(EB)[np.arange(P)[:, None] + P * np.arange(BC)[None]])
    if include_bu:
        shared["bu"] = np.ascontiguousarray(bu).astype(ml_dtypes.bfloat16)

    in_maps = []
    for c in range(N_CORES):
        sl = slice(c * TC, (c + 1) * TC)
        in_maps.append(dict(xh=np.ascontiguousarray(xht_full[:, sl]),
                            xl=np.ascontiguousarray(xlt_full[:, sl]),
                            **shared))
    kernel.last_in_maps = in_maps
    res = run_bass_kernel_spmd(nc, in_maps, core_ids=list(range(N_CORES)))
    out = np.concatenate([res.results[c]["out"].astype(np.float32)
                          for c in range(N_CORES)], axis=0)
    return out.reshape(B_DIM, S_DIM, D)


# revision 11
# speedup vs baseline: 1.3984x; 1.1995x over previous
"""MoE block (AdaptFormer adapters, top-2 of 8 experts) on 8 TRN2 NeuronCores.

Data-parallel over the 8192 tokens (1024/core), router + expert adapter
weights replicated. Per core, one fused streaming pipeline (no phase split):

  - x ships as an exact bf16 hi/lo split, pre-transposed on the host to
    [D, T]; DMA is a fully serialized priority chain (xh block 0 first)
    so early compute is never starved by concurrent transfers.
  - logits per 512-token block: two accumulation groups -> lt_ps [40, 512]:
    rows 0:16 = xh @ [wgh|wgl] (one 16-col pass), rows 32:40 = xl @ wgh.
    Error ~3e-6, far below the 3.6e-5 min top-2/3 logit gap.
  - gating is batched per block: 4 PE transposes land the 40 logit rows in
    one [128, 4, 40] psum tile; the top-2 softmax (x0.5 adapter scale
    folded in) runs as ~16 DVE/ACT ops on [128, 4, 8] strided views
    (broadcast_to for the per-token max/denominator), then 4 PE
    transposes produce g2T [8, 512] bf16.
  - experts run densely in bf16 (error budget 2e-2; bf16 lands ~4e-3):
    HT chunks = Wd^T x -> relu (bf16), GB = Eblk^T @ g2T expands gates
    across the 512-wide expert axis, hg = relu * GB (bf16), out tiles =
    hg @ Wu accumulated over the expert axis, stored bf16, widened on
    the host.
All experts computed densely; sparse gates zero the non-top-2 terms
(mathematically identical to dispatch/combine).
"""
import numpy as np
import ml_dtypes
from contextlib import ExitStack

import concourse.bass as bass
import concourse.tile as tile
from concourse.tile import add_dep_helper
from concourse import bacc, mybir
from concourse.bass_utils import run_bass_kernel_spmd

N_CORES = 8
B_DIM, S_DIM, D = 2, 4096, 1024
T = B_DIM * S_DIM          # 8192 tokens
TC = T // N_CORES          # 1024 tokens per core
E, BK = 8, 64              # experts, bottleneck
EB = E * BK                # 512 concatenated expert axis
P = 128
KC = D // P                # D chunks
HC = KC // 2               # half of the D chunks (split xh block-0 DMA)
BC = EB // P               # bottleneck chunks
LBLK = 512                 # token block
NLB = TC // LBLK           # 2 blocks per core
TPB = LBLK // P            # token tiles per block
SCALE = 0.5
N_WARM = 4                 # PE warm-up matmuls during initial DMA wait
N_FILL = 4                 # PE fillers while wd streams in
G = 2 * E                  # 16 = width of the combined [wgh|wgl] pass
XB = 32                    # xl-pass rows base (matmul out partition 0/32/64)
LW = XB + E                # 40 = logit psum rows (combined + xl pass)

F32 = mybir.dt.float32
F32R = mybir.dt.float32r
BF16 = mybir.dt.bfloat16
AL = mybir.AluOpType
ACTF = mybir.ActivationFunctionType
AX = mybir.AxisListType

_BUILD_CACHE = {}


def _build(include_bd: bool, include_bu: bool, reps: int = 1):
    key = (include_bd, include_bu, reps)
    if key in _BUILD_CACHE:
        return _BUILD_CACHE[key]

    nc = bacc.Bacc("TRN2", target_bir_lowering=False, debug=False,
                   num_devices=N_CORES)
    # x halves, pre-transposed: [D, TC] bf16
    xh_d = nc.dram_tensor("xh", [D, TC], BF16, kind="ExternalInput").ap()
    xl_d = nc.dram_tensor("xl", [D, TC], BF16, kind="ExternalInput").ap()
    wd_d = nc.dram_tensor("wd", [D, EB], BF16, kind="ExternalInput").ap()
    wu_d = nc.dram_tensor("wu", [EB, D], BF16, kind="ExternalInput").ap()
    # [wgh | wgl] side by side: [D, 16] bf16
    wg_d = nc.dram_tensor("wghl", [D, G], BF16, kind="ExternalInput").ap()
    id_d = nc.dram_tensor("ident", [P, P], F32, kind="ExternalInput").ap()
    idb_d = nc.dram_tensor("identb", [P, P], BF16, kind="ExternalInput").ap()
    eb_d = nc.dram_tensor("eblk", [E, EB], BF16, kind="ExternalInput").ap()
    if include_bd:
        bd_d = nc.dram_tensor("bd", [P, BC], F32, kind="ExternalInput").ap()
    if include_bu:
        bu_d = nc.dram_tensor("bu", [E, D], BF16, kind="ExternalInput").ap()
    out_d = nc.dram_tensor("out", [TC, D], BF16, kind="ExternalOutput").ap()

    with tile.TileContext(nc) as tc, ExitStack() as ctx:
        wpool = ctx.enter_context(tc.tile_pool(name="weights", bufs=1))
        hgpool = ctx.enter_context(tc.tile_pool(name="hg", bufs=8))
        rpool = ctx.enter_context(tc.tile_pool(name="relu", bufs=8))
        gpool = ctx.enter_context(tc.tile_pool(name="gates", bufs=2))
        opool = ctx.enter_context(tc.tile_pool(name="osb", bufs=3))

        htgb_ps_pool = ctx.enter_context(
            tc.tile_pool(name="htgb", bufs=2, space="PSUM"))
        lt_ps_pool = ctx.enter_context(
            tc.tile_pool(name="ltps", bufs=2, space="PSUM"))
        small_ps_pool = ctx.enter_context(
            tc.tile_pool(name="smps", bufs=1, space="PSUM"))
        g2t_ps_pool = ctx.enter_context(
            tc.tile_pool(name="g2tps", bufs=1, space="PSUM"))
        o_ps_pool = ctx.enter_context(
            tc.tile_pool(name="ops", bufs=2, space="PSUM"))

        # PE warm-up first: source tile is memset (no DMA wait); keeps the
        # HAM clock un-throttled while the first block's inputs stream in.
        warm32 = wpool.tile([P, LBLK], F32, tag="warm32")
        nc.vector.memset(warm32[:], 0.001)
        warm_src = wpool.tile([P, LBLK], BF16, tag="warmsrc")
        nc.vector.tensor_copy(warm_src[:], warm32[:])
        warm_ps = o_ps_pool.tile([P, LBLK], F32, tag="ops")

        def emit_warm(n):
            for _ in range(n):
                nc.tensor.matmul(warm_ps[:], warm_src[:, 0:P], warm_src[:],
                                 start=True, stop=True)

        emit_warm(N_WARM)

        # ---- priority DMA ladder on the sync queue: each transfer waits
        # for the one two back, keeping ~2 in flight (full serialization
        # costs ~2us handoff per transfer; free-for-all loses priority) ----
        hist = []

        def dma(dst, src):
            i = nc.sync.dma_start(dst, src)
            if len(hist) >= 2:
                add_dep_helper(i.ins, hist[-2].ins, sync=True,
                               reason="dma priority ladder")
            hist.append(i)
            return i

        wg_sb = wpool.tile([P, KC, G], BF16, tag="wghl")
        dma(wg_sb[:], wg_d.rearrange("(c p) n -> p c n", p=P))

        xh_r = xh_d.rearrange("(c p) t -> p c t", p=P)
        xl_r = xl_d.rearrange("(c p) t -> p c t", p=P)
        # xh per (block, half-of-D): 4 tiles so compute starts on the
        # first 512KB
        xh_sb = [[wpool.tile([P, HC, LBLK], BF16, tag=f"xh{b}{h}",
                             name=f"xh{b}{h}") for h in range(2)]
                 for b in range(NLB)]
        xl_sb = [wpool.tile([P, KC, LBLK], BF16, tag=f"xl{b}", name=f"xl{b}")
                 for b in range(NLB)]

        def xh_c(b, c):
            return xh_sb[b][c // HC][:, c % HC, :]

        dma(xh_sb[0][0][:], xh_r[:, 0:HC, 0:LBLK])
        dma(xh_sb[0][1][:], xh_r[:, HC:KC, 0:LBLK])
        wd_sb = wpool.tile([P, KC, EB], BF16, tag="wd")
        dma(wd_sb[:], wd_d.rearrange("(c p) n -> p c n", p=P))
        dma(xl_sb[0][:], xl_r[:, :, 0:LBLK])
        ident = wpool.tile([P, P], F32, tag="ident")
        dma(ident[:], id_d)
        ident_b = wpool.tile([P, P], BF16, tag="identb")
        dma(ident_b[:], idb_d)
        eblk = wpool.tile([E, EB], BF16, tag="eblk")
        dma(eblk[:], eb_d)
        if include_bd:
            bd_sb = wpool.tile([P, BC], F32, tag="bd")
            dma(bd_sb[:], bd_d)
        if include_bu:
            bu_sb = wpool.tile([E, D], BF16, tag="bu")
            dma(bu_sb[:], bu_d)
        dma(xh_sb[1][0][:], xh_r[:, 0:HC, LBLK:TC])
        dma(xh_sb[1][1][:], xh_r[:, HC:KC, LBLK:TC])
        dma(xl_sb[1][:], xl_r[:, :, LBLK:TC])
        wu_sb = wpool.tile([P, BC, D], BF16, tag="wu")
        dma(wu_sb[:], wu_d.rearrange("(c p) n -> p c n", p=P))

        def emit_logits_c(blk):
            """Combined [wgh|wgl] pass -> lt_ps rows 0:16."""
            lt_ps = lt_ps_pool.tile([LW, LBLK], F32, tag="ltps",
                                    name=f"lt{blk}")
            for c in range(KC):
                nc.tensor.matmul(lt_ps[0:G, :], wg_sb[:, c, :], xh_c(blk, c),
                                 start=(c == 0), stop=(c == KC - 1))
            return lt_ps

        def emit_logits_xl(blk, lt_ps):
            """xl @ wgh pass -> lt_ps rows 32:40, then copy to SBUF."""
            for c in range(KC):
                nc.tensor.matmul(lt_ps[XB:LW, :], wg_sb[:, c, 0:E],
                                 xl_sb[blk][:, c, :],
                                 start=(c == 0), stop=(c == KC - 1))
            lt_sb = gpool.tile([LW, LBLK], F32, tag="ltsb")
            nc.scalar.copy(lt_sb[:], lt_ps[:])
            return lt_sb

        def emit_ltT(lt_sb):
            """4 transposes: logit rows for the whole block into PSUM."""
            small = small_ps_pool.tile([P, TPB, LW + 8], F32, tag="smps")
            for t in range(TPB):
                nc.tensor.transpose(small[:, t, 0:LW],
                                    lt_sb[:, bass.ts(t, P)],
                                    ident[0:LW, 0:LW])
            return small

        def emit_chain(small, blk):
            """Batched top-2 softmax (x0.5) for all 512 tokens of a block."""
            l24 = gpool.tile([P, TPB, LW], F32, tag="l24")
            nc.scalar.copy(l24[:], small[:, :, 0:LW])
            l_s = gpool.tile([P, TPB, E], F32, tag="lpart")
            nc.vector.tensor_tensor(l_s[:], l24[:, :, 0:E], l24[:, :, E:G],
                                    op=AL.add)
            l_sb = gpool.tile([P, TPB, E], F32, tag="lsb")
            nc.vector.tensor_tensor(l_sb[:], l_s[:], l24[:, :, XB:LW],
                                    op=AL.add)
            sh3 = [P, TPB, E]
            m1 = gpool.tile([P, TPB, 1], F32, tag="m1")
            nc.vector.tensor_reduce(m1[:, :, 0], l_sb[:], AX.X, AL.max)
            mask1 = gpool.tile(sh3, F32, tag="mask1")
            nc.vector.tensor_tensor(mask1[:], l_sb[:],
                                    m1[:].broadcast_to(sh3), op=AL.is_ge)
            lm = gpool.tile(sh3, F32, tag="lm")
            nc.vector.scalar_tensor_tensor(
                lm[:], mask1[:], -1e30, l_sb[:], op0=AL.mult, op1=AL.add)
            m2 = gpool.tile([P, TPB, 1], F32, tag="m2")
            nc.vector.tensor_reduce(m2[:, :, 0], lm[:], AX.X, AL.max)
            e2m = gpool.tile([P, TPB, 1], F32, tag="e2m")
            nc.vector.tensor_tensor(e2m[:], m2[:], m1[:], op=AL.subtract)
            e2 = gpool.tile([P, TPB, 1], F32, tag="e2")
            nc.scalar.activation(e2[:], e2m[:], ACTF.Exp)
            d2 = gpool.tile([P, TPB, 1], F32, tag="d2")
            nc.scalar.activation(d2[:], e2[:], ACTF.Copy,
                                 bias=1.0 / SCALE, scale=1.0 / SCALE)
            rh = gpool.tile([P, TPB, 1], F32, tag="rh")
            nc.vector.reciprocal(rh[:], d2[:])
            lsh = gpool.tile(sh3, F32, tag="lsh")
            nc.vector.tensor_tensor(lsh[:], l_sb[:],
                                    m1[:].broadcast_to(sh3), op=AL.subtract)
            expl = gpool.tile(sh3, F32, tag="expl")
            nc.scalar.activation(expl[:], lsh[:], ACTF.Exp)
            mask2 = gpool.tile(sh3, F32, tag="mask2")
            nc.vector.tensor_tensor(mask2[:], l_sb[:],
                                    m2[:].broadcast_to(sh3), op=AL.is_ge)
            t1 = gpool.tile(sh3, F32, tag="t1")
            nc.vector.tensor_tensor(t1[:], expl[:], mask2[:], op=AL.mult)
            g2 = gpool.tile(sh3, BF16, tag="g2", name=f"g2_{blk}")
            nc.vector.tensor_tensor(g2[:], t1[:],
                                    rh[:].broadcast_to(sh3), op=AL.mult)
            return g2

        def emit_g2T(g2):
            """4 transposes: gates back to [8, tok] bf16 in SBUF."""
            g2t_ps = g2t_ps_pool.tile([E, LBLK], BF16, tag="g2tps")
            for t in range(TPB):
                nc.tensor.transpose(g2t_ps[:, bass.ts(t, P)], g2[:, t, :],
                                    ident_b[:])
            g2t_sb = gpool.tile([E, LBLK], BF16, tag="g2t")
            nc.scalar.copy(g2t_sb[:], g2t_ps[:])
            return g2t_sb

        def emit_ht(blk, k):
            """HT chunk k: relu(Wd^T x) in bf16."""
            ht_ps = htgb_ps_pool.tile([P, LBLK], F32, tag="htps")
            for c in range(KC):
                nc.tensor.matmul(ht_ps[:], wd_sb[:, c, bass.ts(k, P)],
                                 xh_c(blk, c),
                                 start=(c == 0), stop=(c == KC - 1))
            r_k = rpool.tile([P, LBLK], BF16, tag="relu")
            if include_bd:
                nc.scalar.activation(r_k[:], ht_ps[:], ACTF.Relu,
                                     bias=bd_sb[:, k:k + 1])
            else:
                nc.scalar.activation(r_k[:], ht_ps[:], ACTF.Relu)
            return r_k

        def emit_gb(k, g2t_sb):
            """Gate-expand matmul for chunk k."""
            gb_ps = htgb_ps_pool.tile([P, LBLK], F32, tag="htps")
            nc.tensor.matmul(gb_ps[:], eblk[:, bass.ts(k, P)], g2t_sb[:],
                             start=True, stop=True)
            return gb_ps

        def emit_hg(blk, k, r_k, gb_ps):
            """hg = relu * gates (bf16, DVE)."""
            hg_k = hgpool.tile([P, LBLK], BF16, tag="hg",
                               name=f"hg{blk}_{k}")
            nc.vector.tensor_tensor(hg_k[:], r_k[:], gb_ps[:], op=AL.mult)
            return hg_k

        def emit_out(blk, hgs, g2t_sb):
            """out tiles = HG @ Wu (+ g2 @ bu), stored bf16."""
            for bo in range(TPB):
                t = blk * TPB + bo
                rows = bass.ts(t, P)
                tok = bass.ts(bo, P)
                for h in range(2):
                    o_ps = o_ps_pool.tile([P, 512], F32, tag="ops")
                    for k in range(BC):
                        nc.tensor.matmul(
                            o_ps[:], hgs[k][:, tok],
                            wu_sb[:, k, bass.ts(h, 512)],
                            start=(k == 0),
                            stop=(k == BC - 1 and not include_bu))
                    if include_bu:
                        nc.tensor.matmul(o_ps[:], g2t_sb[:, tok],
                                         bu_sb[:, bass.ts(h, 512)],
                                         start=False, stop=True)
                    o_sb = opool.tile([P, 512], BF16, tag="osb")
                    if h == 0:
                        nc.vector.tensor_copy(o_sb[:], o_ps[:])
                    else:
                        nc.scalar.copy(o_sb[:], o_ps[:])
                    nc.scalar.dma_start(out_d[rows, bass.ts(h, 512)],
                                        o_sb[:])

        for rep in range(reps):
            # ---- block 0: logits stream behind the xh DMA; the gating
            # chain (DVE/ACT) hides under HT matmuls ----
            lt0 = emit_logits_c(0)
            emit_warm(N_FILL)
            r0 = [emit_ht(0, k) for k in range(2)]
            lt_sb0 = emit_logits_xl(0, lt0)
            small0 = emit_ltT(lt_sb0)
            g2_0 = emit_chain(small0, 0)
            r0 += [emit_ht(0, k) for k in range(2, BC)]
            g2t0 = emit_g2T(g2_0)
            gbs0 = [emit_gb(k, g2t0) for k in range(BC)]
            hgs0 = [emit_hg(0, k, r0[k], gbs0[k]) for k in range(BC)]

            # ---- block 1 gating; its chain hides under HT1/OUT0 ----
            lt1 = emit_logits_c(1)
            lt_sb1 = emit_logits_xl(1, lt1)
            small1 = emit_ltT(lt_sb1)
            g2_1 = emit_chain(small1, 1)
            r1 = [emit_ht(1, k) for k in range(BC)]
            emit_out(0, hgs0, g2t0)
            g2t1 = emit_g2T(g2_1)
            gbs1 = [emit_gb(k, g2t1) for k in range(BC)]
            hgs1 = [emit_hg(1, k, r1[k], gbs1[k]) for k in range(BC)]
            emit_out(1, hgs1, g2t1)

    nc.compile()
    _BUILD_CACHE[key] = nc
    return nc


def _split_bf16(a):
    hi = a.astype(ml_dtypes.bfloat16)
    lo = (a - hi.astype(np.float32)).astype(ml_dtypes.bfloat16)
    return hi, lo


def kernel(x, w_gate, w_noise, Wd, bd, Wu, bu, reps: int = 1):
    x = np.ascontiguousarray(np.asarray(x, dtype=np.float32))
    assert x.shape == (B_DIM, S_DIM, D), x.shape
    wg = np.ascontiguousarray(np.asarray(w_gate, dtype=np.float32))
    Wd = np.asarray(Wd, dtype=np.float32)
    Wu = np.asarray(Wu, dtype=np.float32)
    bd = np.asarray(bd, dtype=np.float32)
    bu = np.asarray(bu, dtype=np.float32)

    include_bd = bool(np.any(bd))
    include_bu = bool(np.any(bu))
    nc = _build(include_bd, include_bu, reps)

    xf = x.reshape(T, D)
    xh, xl = _split_bf16(xf)
    xht_full = np.ascontiguousarray(xh.T)   # [D, T]
    xlt_full = np.ascontiguousarray(xl.T)
    wgh, wgl = _split_bf16(wg)
    wghl = np.ascontiguousarray(
        np.concatenate([wgh, wgl], axis=1))            # [D, 16] bf16
    wd_all = np.ascontiguousarray(
        Wd.transpose(1, 0, 2).reshape(D, EB)).astype(ml_dtypes.bfloat16)
    wu_flat = np.ascontiguousarray(
        Wu.reshape(EB, D)).astype(ml_dtypes.bfloat16)
    ident = np.eye(P, dtype=np.float32)
    eblk = np.kron(np.eye(E, dtype=np.float32),
                   np.ones((1, BK), dtype=np.float32))  # [E, EB]

    shared = dict(wd=wd_all, wu=wu_flat, wghl=wghl, ident=ident,
                  identb=ident.astype(ml_dtypes.bfloat16),
                  eblk=eblk.astype(ml_dtypes.bfloat16))
    if include_bd:
        # [P, BC] partition-major per chunk: bd_sb[p, k] = bd_flat[128k+p]
        shared["bd"] = np.ascontiguousarray(
            bd.reshape(EB)[np.arange(P)[:, None] + P * np.arange(BC)[None]])
    if include_bu:
        shared["bu"] = np.ascontiguousarray(bu).astype(ml_dtypes.bfloat16)

    in_maps = []
    for c in range(N_CORES):
        sl = slice(c * TC, (c + 1) * TC)
        in_maps.append(dict(xh=np.ascontiguousarray(xht_full[:, sl]),
                            xl=np.ascontiguousarray(xlt_full[:, sl]),
                            **shared))
    kernel.last_in_maps = in_maps
    res = run_bass_kernel_spmd(nc, in_maps, core_ids=list(range(N_CORES)))
    out = np.concatenate([res.results[c]["out"].astype(np.float32)
                          for c in range(N_CORES)], axis=0)
    return out.reshape(B_DIM, S_DIM, D)
